# revision 6
# baseline (speedup 1.0000x reference)
"""nn_LocalInference_58695023067411: batch-parallel Bass/Tile kernel, 8 NeuronCores.

Math per batch element (B=8, L=2048, D=128, one core per batch element):
  s  = a @ b.T                      # [L, L]
  a_ = softmax(s, axis=1) @ b       # row softmax
  b_ = softmax(s, axis=0) @ a       # col softmax
  out = [[a, a_, a-a_, a*a_], [b, b_, b-b_, b*b_]]   # [2, L, 4D]

Kernel strategy (per core):
  * Everything is computed from ET[j,i] = exp(s[i,j] - 50).  The constant
    shift is softmax-invariant on both axes and keeps exp() comfortably
    inside f32/bf16 range (max |s| ~ 84 on these inputs).
  * Phase 1: ET = exp(bT.T @ aT - 50) via PE matmuls (bf16 in, f32 PSUM),
    ScalarE exp PSUM->SBUF(bf16) with accum_out giving colsum_j for free.
  * Phase 2: one fused matmul per 128-row output chunk:
      psum[i, 0:129+D] = sum_j ET[j,i] * [b | ones | a/colsum]_j
    giving unnormalized a_, rowsum_i, and b_ in a single pass with ET as
    the stationary operand (loaded into the PE exactly once).
  * Epilogue: a_ = psum * (1/rowsum) (ScalarE), derived ma/mb columns on
    VectorE, raw a/b columns of the output written straight from the input
    tiles with one big DMA each.

All matmul operands are bf16 (1 PE cycle/row vs 4 for f32); measured
end-to-end rel err vs the f32 reference is ~6e-3 (gate: 2e-2).
"""

import os
import sys

import numpy as np

sys.path.insert(0, "/opt/trn_rl_repo")

B, L, D = 8, 2048, 128
P = 128
NT = L // P          # 16 row/col chunks of 128
C_SHIFT = 50.0       # subtracted inside exp; softmax-shift-invariant

_CACHE = {}


def _emit(ctx, tc, nc, a_dram, b_dram, o_dram):
    import concourse.mybir as mybir
    from concourse.masks import make_identity

    f32 = mybir.dt.float32
    bf16 = mybir.dt.bfloat16
    Exp = mybir.ActivationFunctionType.Exp
    Copy = mybir.ActivationFunctionType.Copy

    persist = ctx.enter_context(tc.tile_pool(name="persist", bufs=1))
    et_pool = ctx.enter_context(tc.tile_pool(name="et", bufs=NT))
    rhs_pool = ctx.enter_context(tc.tile_pool(name="rhs", bufs=NT))
    stats = ctx.enter_context(tc.tile_pool(name="stats", bufs=4))
    out_pool = ctx.enter_context(tc.tile_pool(name="outp", bufs=4))
    psA = ctx.enter_context(tc.tile_pool(name="psA", bufs=2, space="PSUM"))
    psB = ctx.enter_context(tc.tile_pool(name="psB", bufs=4, space="PSUM"))

    # ---- load inputs: natural layout [p, t, d] with p the fast row index
    a_nat = persist.tile([P, NT, D], f32)
    b_nat = persist.tile([P, NT, D], f32)
    nc.sync.dma_start(out=a_nat, in_=a_dram.rearrange("(t p) d -> p t d", p=P))
    nc.sync.dma_start(out=b_nat, in_=b_dram.rearrange("(t p) d -> p t d", p=P))

    # raw a/b occupy out[:, :, 0:D]; write them straight from the input tiles
    nc.sync.dma_start(
        out=o_dram[0, :, 0:D].rearrange("(t p) d -> p t d", p=P), in_=a_nat
    )
    nc.sync.dma_start(
        out=o_dram[1, :, 0:D].rearrange("(t p) d -> p t d", p=P), in_=b_nat
    )

    # ---- transpose a, b to [d, i] layout (bf16) via PE transposes
    ident = persist.tile([P, P], f32)
    make_identity(nc, ident)
    neg_shift = persist.tile([P, 1], f32)
    nc.vector.memset(neg_shift, -C_SHIFT)
    aT = persist.tile([P, L], bf16)
    bT = persist.tile([P, L], bf16)
    for src, dstT in ((a_nat, aT), (b_nat, bT)):
        for g in range(2):
            ps = psA.tile([P, 8 * P], f32, tag="p1")
            for k in range(8):
                t = g * 8 + k
                nc.tensor.matmul(
                    ps[:, k * P : (k + 1) * P],
                    lhsT=src[:, t, :],
                    rhs=ident,
                    is_transpose=True,
                    start=True,
                    stop=True,
                )
            nc.vector.tensor_copy(out=dstT[:, g * 1024 : (g + 1) * 1024], in_=ps)

    # ---- phase 1: ET_j = exp(s^T chunk) + colsum via accum, rhs tiles
    N2 = 2 * D + 1  # [b | ones | a/colsum]
    ets = []
    rhss = []
    for jt in range(NT):
        et_t = et_pool.tile([P, L], bf16, tag="et")
        cs2 = stats.tile([P, 2], f32)
        for h in range(2):
            ps = psA.tile([P, 1024], f32, tag="p1")
            for q in range(2):
                isl = h * 2 + q
                nc.tensor.matmul(
                    ps[:, q * 512 : (q + 1) * 512],
                    lhsT=bT[:, jt * P : (jt + 1) * P],
                    rhs=aT[:, isl * 512 : (isl + 1) * 512],
                    start=True,
                    stop=True,
                )
            nc.scalar.activation(
                out=et_t[:, h * 1024 : (h + 1) * 1024],
                in_=ps,
                func=Exp,
                bias=neg_shift,
                scale=1.0,
                accum_out=cs2[:, h : h + 1],
            )
        csum = stats.tile([P, 1], f32)
        rcol = stats.tile([P, 1], f32)
        nc.vector.tensor_add(csum, cs2[:, 0:1], cs2[:, 1:2])
        nc.vector.reciprocal(rcol, csum)
        rhs_t = rhs_pool.tile([P, N2], bf16, tag="rhs")
        nc.vector.tensor_copy(out=rhs_t[:, 0:D], in_=b_nat[:, jt, :])
        nc.vector.memset(rhs_t[:, D : D + 1], 1.0)
        nc.vector.tensor_scalar_mul(
            out=rhs_t[:, D + 1 : N2], in0=a_nat[:, jt, :], scalar1=rcol
        )
        ets.append(et_t)
        rhss.append(rhs_t)

    # ---- phase 2: fused [a_unnorm | rowsum | b_] matmul + epilogue per chunk
    for it in range(NT):
        po = psB.tile([P, N2], f32, tag="p2")
        for jt in range(NT):
            nc.tensor.matmul(
                po,
                lhsT=ets[jt][:, it * P : (it + 1) * P],
                rhs=rhss[jt],
                start=(jt == 0),
                stop=(jt == NT - 1),
            )
        rrow = stats.tile([P, 1], f32)
        nc.vector.reciprocal(rrow, po[:, D : D + 1])
        ma_t = out_pool.tile([P, 3 * D], f32, tag="ma")
        mb_t = out_pool.tile([P, 3 * D], f32, tag="mb")
        # a_ = a_unnorm / rowsum ; b_ comes out normalized already
        nc.scalar.activation(out=ma_t[:, 0:D], in_=po[:, 0:D], func=Copy, bias=0.0, scale=rrow)
        nc.scalar.activation(out=mb_t[:, 0:D], in_=po[:, D + 1 : N2], func=Copy, bias=0.0, scale=1.0)
        nc.vector.tensor_sub(ma_t[:, D : 2 * D], a_nat[:, it, :], ma_t[:, 0:D])
        nc.vector.tensor_mul(ma_t[:, 2 * D : 3 * D], a_nat[:, it, :], ma_t[:, 0:D])
        nc.vector.tensor_sub(mb_t[:, D : 2 * D], b_nat[:, it, :], mb_t[:, 0:D])
        nc.vector.tensor_mul(mb_t[:, 2 * D : 3 * D], b_nat[:, it, :], mb_t[:, 0:D])
        nc.sync.dma_start(out=o_dram[0, it * P : (it + 1) * P, D : 4 * D], in_=ma_t)
        nc.sync.dma_start(out=o_dram[1, it * P : (it + 1) * P, D : 4 * D], in_=mb_t)


def _build_nc():
    import concourse.bacc as bacc
    import concourse.mybir as mybir
    import concourse.tile as tile

    f32 = mybir.dt.float32
    nc = bacc.Bacc("TRN2", target_bir_lowering=False, debug=False, num_devices=B)
    a_dram = nc.dram_tensor("a", [L, D], f32, kind="ExternalInput").ap()
    b_dram = nc.dram_tensor("b", [L, D], f32, kind="ExternalInput").ap()
    o_dram = nc.dram_tensor("o", [2, L, 4 * D], f32, kind="ExternalOutput").ap()
    from contextlib import ExitStack

    with tile.TileContext(nc) as tc:
        with ExitStack() as ctx:
            _emit(ctx, tc, nc, a_dram, b_dram, o_dram)
    nc.finalize()
    return nc


def _get_nc():
    if "nc" not in _CACHE:
        _CACHE["nc"] = _build_nc()
    return _CACHE["nc"]


def kernel(a: np.ndarray, b: np.ndarray) -> np.ndarray:
    """Full inputs [8, 2048, 128] f32 -> full output [2, 8, 2048, 512] f32."""
    a = np.ascontiguousarray(a, dtype=np.float32)
    b = np.ascontiguousarray(b, dtype=np.float32)
    nc = _get_nc()
    from concourse import bass_utils

    in_maps = [{"a": a[c], "b": b[c]} for c in range(B)]
    res = bass_utils.run_bass_kernel_spmd(nc, in_maps, core_ids=list(range(B)))
    out = np.empty((2, B, L, 4 * D), dtype=np.float32)
    for c in range(B):
        out[:, c] = res.results[c]["o"]
    return out


# revision 7
# speedup vs baseline: 1.0219x; 1.0219x over previous
"""nn_LocalInference_58695023067411: batch-parallel Bass/Tile kernel, 8 NeuronCores.

Math per batch element (B=8, L=2048, D=128, one core per batch element):
  s  = a @ b.T                      # [L, L]
  a_ = softmax(s, axis=1) @ b       # row softmax
  b_ = softmax(s, axis=0) @ a       # col softmax
  out = [[a, a_, a-a_, a*a_], [b, b_, b-b_, b*b_]]   # [2, L, 4D]

Kernel strategy (per core):
  * Everything is computed from ET[j,i] = exp(s[i,j] - 50).  The constant
    shift is softmax-invariant on both axes and keeps exp() comfortably
    inside f32/bf16 range (max |s| ~ 84 on these inputs).
  * Phase 1: ET = exp(bT.T @ aT - 50) via PE matmuls (bf16 in, f32 PSUM),
    ScalarE exp PSUM->SBUF(bf16) with accum_out giving colsum_j for free.
    ScalarE is the pacing engine (~39us of exp); loads/transposes are
    ordered so the first exp can issue at ~4us.
  * Phase 2: one fused matmul per 128-row output chunk:
      psum[i, 0:2D+1] = sum_j ET[j,i] * [b | ones | a/colsum]_j
    giving unnormalized a_, rowsum_i, and b_ in one pass with ET stationary.
    The j-contraction is split in half: first-half partials run on the PE
    during the exp window (PSUM -> SBUF spill), second halves + epilogue
    run in the tail so the serial-after-last-exp work is halved.
  * Raw a/b columns of the output go out as dep-free HBM->HBM DMAs that fill
    the otherwise-idle DMA window during phase 1.

All matmul operands are bf16 (1 PE cycle/row vs 4 for f32); measured
end-to-end rel err vs the f32 reference is ~6e-3 (gate: 2e-2).
"""

import os
import sys

import numpy as np

sys.path.insert(0, "/opt/trn_rl_repo")

B, L, D = 8, 2048, 128
P = 128
NT = L // P          # 16 row/col chunks of 128
NH = NT // 2         # half-split of the phase-2 j contraction
C_SHIFT = 50.0       # subtracted inside exp; softmax-shift-invariant
N2 = 2 * D + 1       # [b | ones | a/colsum]

_CACHE = {}


def _emit(ctx, tc, nc, a_dram, b_dram, o_dram):
    import concourse.mybir as mybir
    from concourse.masks import make_identity

    f32 = mybir.dt.float32
    bf16 = mybir.dt.bfloat16
    Exp = mybir.ActivationFunctionType.Exp
    Copy = mybir.ActivationFunctionType.Copy

    persist = ctx.enter_context(tc.tile_pool(name="persist", bufs=1))
    et_pool = ctx.enter_context(tc.tile_pool(name="et", bufs=NT))
    rhs_pool = ctx.enter_context(tc.tile_pool(name="rhs", bufs=NT))
    part_pool = ctx.enter_context(tc.tile_pool(name="part", bufs=NT))
    stats = ctx.enter_context(tc.tile_pool(name="stats", bufs=4))
    sum_pool = ctx.enter_context(tc.tile_pool(name="sum", bufs=4))
    out_pool = ctx.enter_context(tc.tile_pool(name="outp", bufs=4))
    psA = ctx.enter_context(tc.tile_pool(name="psA", bufs=2, space="PSUM"))
    psB = ctx.enter_context(tc.tile_pool(name="psB", bufs=4, space="PSUM"))

    # ---- constants
    ident = persist.tile([P, P], f32)
    make_identity(nc, ident)
    neg_shift = persist.tile([P, 1], f32)
    nc.vector.memset(neg_shift, -C_SHIFT)

    # ---- load inputs: natural layout [p, t, d], split in halves so the
    # transposes (and the first exps) start as early as possible.
    a_nat = persist.tile([P, NT, D], f32)
    b_nat = persist.tile([P, NT, D], f32)
    a_v = a_dram.rearrange("(t p) d -> p t d", p=P)
    b_v = b_dram.rearrange("(t p) d -> p t d", p=P)
    nc.sync.dma_start(out=b_nat[:, 0:NH, :], in_=b_v[:, 0:NH, :])
    nc.sync.dma_start(out=a_nat[:, 0:NH, :], in_=a_v[:, 0:NH, :])
    nc.sync.dma_start(out=a_nat[:, NH:NT, :], in_=a_v[:, NH:NT, :])
    nc.sync.dma_start(out=b_nat[:, NH:NT, :], in_=b_v[:, NH:NT, :])

    # raw a/b occupy out[:, :, 0:D]; dep-free HBM->HBM copies that fill the
    # DMA-idle window during phase 1.
    nc.sync.dma_start(out=o_dram[0, :, 0:D], in_=a_dram)
    nc.sync.dma_start(out=o_dram[1, :, 0:D], in_=b_dram)

    # ---- transpose a, b to [d, i] layout (bf16) via PE transposes.
    # Order: b-half0 (lhsT of the first 8 j-chunks), then a (rhs of all),
    # then b-half1.
    aT = persist.tile([P, L], bf16)
    bT = persist.tile([P, L], bf16)
    for src, dstT, g in ((b_nat, bT, 0), (a_nat, aT, 0), (a_nat, aT, 1), (b_nat, bT, 1)):
        ps = psA.tile([P, 8 * P], f32, tag="p1")
        for k in range(8):
            t = g * 8 + k
            nc.tensor.matmul(
                ps[:, k * P : (k + 1) * P],
                lhsT=src[:, t, :],
                rhs=ident,
                is_transpose=True,
                start=True,
                stop=True,
            )
        nc.vector.tensor_copy(out=dstT[:, g * 1024 : (g + 1) * 1024], in_=ps)

    # ---- phase 1: ET_j = exp(s^T chunk) + colsum via accum, rhs tiles
    ets = []
    rhss = []

    def phase1(jt):
        et_t = et_pool.tile([P, L], bf16, tag="et")
        cs2 = stats.tile([P, 2], f32)
        for h in range(2):
            ps = psA.tile([P, 1024], f32, tag="p1")
            for q in range(2):
                isl = h * 2 + q
                nc.tensor.matmul(
                    ps[:, q * 512 : (q + 1) * 512],
                    lhsT=bT[:, jt * P : (jt + 1) * P],
                    rhs=aT[:, isl * 512 : (isl + 1) * 512],
                    start=True,
                    stop=True,
                )
            nc.scalar.activation(
                out=et_t[:, h * 1024 : (h + 1) * 1024],
                in_=ps,
                func=Exp,
                bias=neg_shift,
                scale=1.0,
                accum_out=cs2[:, h : h + 1],
            )
        csum = stats.tile([P, 1], f32)
        rcol = stats.tile([P, 1], f32)
        nc.vector.tensor_add(csum, cs2[:, 0:1], cs2[:, 1:2])
        nc.vector.reciprocal(rcol, csum)
        rhs_t = rhs_pool.tile([P, N2], bf16, tag="rhs")
        nc.vector.tensor_copy(out=rhs_t[:, 0:D], in_=b_nat[:, jt, :])
        nc.vector.memset(rhs_t[:, D : D + 1], 1.0)
        nc.vector.tensor_scalar_mul(
            out=rhs_t[:, D + 1 : N2], in0=a_nat[:, jt, :], scalar1=rcol
        )
        ets.append(et_t)
        rhss.append(rhs_t)

    for jt in range(NT):
        phase1(jt)

    # ---- phase 2a: first-half partials (j chunks 0..NH-1), spilled to SBUF.
    # Emitted after the full phase-1 loop so phase-1 matmuls keep PE priority;
    # these fill PE idle time while ScalarE works through the exps.
    parts = []
    for it in range(NT):
        po = psB.tile([P, N2], f32, tag="p2")
        for jt in range(NH):
            nc.tensor.matmul(
                po,
                lhsT=ets[jt][:, it * P : (it + 1) * P],
                rhs=rhss[jt],
                start=(jt == 0),
                stop=(jt == NH - 1),
            )
        pa = part_pool.tile([P, N2], f32, tag="pa")
        nc.vector.tensor_copy(out=pa, in_=po)
        parts.append(pa)

    # ---- phase 2b: second-half accumulation + epilogue per chunk
    for it in range(NT):
        po = psB.tile([P, N2], f32, tag="p2")
        for jt in range(NH, NT):
            nc.tensor.matmul(
                po,
                lhsT=ets[jt][:, it * P : (it + 1) * P],
                rhs=rhss[jt],
                start=(jt == NH),
                stop=(jt == NT - 1),
            )
        tot = sum_pool.tile([P, N2], f32, tag="tot")
        nc.vector.tensor_add(tot, po, parts[it])
        rrow = stats.tile([P, 1], f32)
        nc.vector.reciprocal(rrow, tot[:, D : D + 1])
        ma_t = out_pool.tile([P, 3 * D], f32, tag="ma")
        mb_t = out_pool.tile([P, 3 * D], f32, tag="mb")
        # a_ = a_unnorm / rowsum ; b_ comes out normalized already
        nc.scalar.activation(out=ma_t[:, 0:D], in_=tot[:, 0:D], func=Copy, bias=0.0, scale=rrow)
        nc.scalar.activation(out=mb_t[:, 0:D], in_=tot[:, D + 1 : N2], func=Copy, bias=0.0, scale=1.0)
        nc.vector.tensor_sub(ma_t[:, D : 2 * D], a_nat[:, it, :], ma_t[:, 0:D])
        nc.vector.tensor_mul(ma_t[:, 2 * D : 3 * D], a_nat[:, it, :], ma_t[:, 0:D])
        nc.vector.tensor_sub(mb_t[:, D : 2 * D], b_nat[:, it, :], mb_t[:, 0:D])
        nc.vector.tensor_mul(mb_t[:, 2 * D : 3 * D], b_nat[:, it, :], mb_t[:, 0:D])
        nc.sync.dma_start(out=o_dram[0, it * P : (it + 1) * P, D : 4 * D], in_=ma_t)
        nc.sync.dma_start(out=o_dram[1, it * P : (it + 1) * P, D : 4 * D], in_=mb_t)


def _build_nc():
    import concourse.bacc as bacc
    import concourse.mybir as mybir
    import concourse.tile as tile

    f32 = mybir.dt.float32
    nc = bacc.Bacc("TRN2", target_bir_lowering=False, debug=False, num_devices=B)
    a_dram = nc.dram_tensor("a", [L, D], f32, kind="ExternalInput").ap()
    b_dram = nc.dram_tensor("b", [L, D], f32, kind="ExternalInput").ap()
    o_dram = nc.dram_tensor("o", [2, L, 4 * D], f32, kind="ExternalOutput").ap()
    from contextlib import ExitStack

    with tile.TileContext(nc) as tc:
        with ExitStack() as ctx:
            _emit(ctx, tc, nc, a_dram, b_dram, o_dram)
    nc.finalize()
    return nc


def _get_nc():
    if "nc" not in _CACHE:
        _CACHE["nc"] = _build_nc()
    return _CACHE["nc"]


def kernel(a: np.ndarray, b: np.ndarray) -> np.ndarray:
    """Full inputs [8, 2048, 128] f32 -> full output [2, 8, 2048, 512] f32."""
    a = np.ascontiguousarray(a, dtype=np.float32)
    b = np.ascontiguousarray(b, dtype=np.float32)
    nc = _get_nc()
    from concourse import bass_utils

    in_maps = [{"a": a[c], "b": b[c]} for c in range(B)]
    res = bass_utils.run_bass_kernel_spmd(nc, in_maps, core_ids=list(range(B)))
    out = np.empty((2, B, L, 4 * D), dtype=np.float32)
    for c in range(B):
        out[:, c] = res.results[c]["o"]
    return out


# revision 9
# speedup vs baseline: 1.1122x; 1.0883x over previous
"""nn_LocalInference_58695023067411: batch-parallel Bass/Tile kernel, 8 NeuronCores.

Math per batch element (B=8, L=2048, D=128, one core per batch element):
  s  = a @ b.T                      # [L, L]
  a_ = softmax(s, axis=1) @ b       # row softmax
  b_ = softmax(s, axis=0) @ a       # col softmax
  out = [[a, a_, a-a_, a*a_], [b, b_, b-b_, b*b_]]   # [2, L, 4D]

Kernel strategy (per core):
  * Everything is computed from ET[j,i] = exp(s[i,j] - 50).  The constant
    shift is softmax-invariant on both axes and keeps exp() comfortably
    inside f32/bf16 range (max |s| ~ 84 on these inputs).
  * Phase 1: ET = exp(bT.T @ aT - 50) via PE matmuls (bf16 in, f32 PSUM),
    ScalarE exp PSUM->SBUF(bf16) with accum_out giving colsum_j for free.
    ScalarE is the pacing engine (~39us of exp); loads/transposes are
    ordered so the first exp can issue at ~4us.
  * Phase 2: one fused matmul per 128-row output chunk:
      psum[i, 0:2D+1] = sum_j ET[j,i] * [b | ones | a/colsum]_j
    giving unnormalized a_, rowsum_i, and b_ in one pass with ET stationary.
    The j-contraction is split in half: first-half partials run on the PE
    during the exp window (PSUM -> SBUF spill), second halves + epilogue
    run in the tail so the serial-after-last-exp work is halved.
  * Raw a/b columns of the output go out as dep-free HBM->HBM DMAs that fill
    the otherwise-idle DMA window during phase 1.

All matmul operands are bf16 (1 PE cycle/row vs 4 for f32); measured
end-to-end rel err vs the f32 reference is ~6e-3 (gate: 2e-2).
"""

import os
import sys

import numpy as np

sys.path.insert(0, "/opt/trn_rl_repo")

B, L, D = 8, 2048, 128
P = 128
NT = L // P          # 16 row/col chunks of 128
NH = NT // 2         # half-split of the phase-2 j contraction
C_SHIFT = 50.0       # subtracted inside exp; softmax-shift-invariant
N2 = 2 * D + 1       # [b | ones | a/colsum]

_CACHE = {}


def _emit(ctx, tc, nc, a_dram, b_dram, o_dram):
    import concourse.mybir as mybir
    from concourse.masks import make_identity

    f32 = mybir.dt.float32
    bf16 = mybir.dt.bfloat16
    Exp = mybir.ActivationFunctionType.Exp
    Copy = mybir.ActivationFunctionType.Copy

    persist = ctx.enter_context(tc.tile_pool(name="persist", bufs=1))
    et_pool = ctx.enter_context(tc.tile_pool(name="et", bufs=NT))
    rhs_pool = ctx.enter_context(tc.tile_pool(name="rhs", bufs=NT))
    part_pool = ctx.enter_context(tc.tile_pool(name="part", bufs=NT))
    stats = ctx.enter_context(tc.tile_pool(name="stats", bufs=4))
    sum_pool = ctx.enter_context(tc.tile_pool(name="sum", bufs=4))
    out_pool = ctx.enter_context(tc.tile_pool(name="outp", bufs=4))
    psA = ctx.enter_context(tc.tile_pool(name="psA", bufs=2, space="PSUM"))
    psB = ctx.enter_context(tc.tile_pool(name="psB", bufs=2, space="PSUM"))
    psT = ctx.enter_context(tc.tile_pool(name="psT", bufs=2, space="PSUM"))

    # ---- constants
    ident = persist.tile([P, P], f32)
    make_identity(nc, ident)
    neg_shift = persist.tile([P, 1], f32)
    nc.vector.memset(neg_shift, -C_SHIFT)

    # ---- load inputs: natural layout [p, t, d], split so the transposes
    # (and the first exps) start as early as possible.
    a_nat = persist.tile([P, NT, D], f32)
    b_nat = persist.tile([P, NT, D], f32)
    a_v = a_dram.rearrange("(t p) d -> p t d", p=P)
    b_v = b_dram.rearrange("(t p) d -> p t d", p=P)
    nc.sync.dma_start(out=b_nat[:, 0:4, :], in_=b_v[:, 0:4, :])
    nc.sync.dma_start(out=a_nat[:, 0:8, :], in_=a_v[:, 0:8, :])
    nc.sync.dma_start(out=b_nat[:, 4:8, :], in_=b_v[:, 4:8, :])
    nc.sync.dma_start(out=a_nat[:, 8:16, :], in_=a_v[:, 8:16, :])
    nc.sync.dma_start(out=b_nat[:, 8:16, :], in_=b_v[:, 8:16, :])

    # raw a/b occupy out[:, :, 0:D]; dep-free HBM->HBM copies that fill the
    # DMA-idle window during phase 1.
    nc.sync.dma_start(out=o_dram[0, :, 0:D], in_=a_dram)
    nc.sync.dma_start(out=o_dram[1, :, 0:D], in_=b_dram)

    # ---- transpose a, b to [d, i] layout (bf16) via PE transposes, in
    # groups of 4 through a dedicated 1-bank psum pool.  Order matches the
    # dependency chain of the first exps: phase1(jt<4) needs bT[0:512] and
    # all of aT used by its first psum half (aT[0:1024]).
    aT = persist.tile([P, L], bf16)
    bT = persist.tile([P, L], bf16)
    tr_order = [
        (b_nat, bT, 0), (a_nat, aT, 0), (a_nat, aT, 1), (b_nat, bT, 1),
        (a_nat, aT, 2), (a_nat, aT, 3), (b_nat, bT, 2), (b_nat, bT, 3),
    ]
    for src, dstT, g in tr_order:
        ps = psT.tile([P, 4 * P], f32, tag="tr")
        for k in range(4):
            t = g * 4 + k
            nc.tensor.matmul(
                ps[:, k * P : (k + 1) * P],
                lhsT=src[:, t, :],
                rhs=ident,
                is_transpose=True,
                start=True,
                stop=True,
            )
        nc.vector.tensor_copy(out=dstT[:, g * 512 : (g + 1) * 512], in_=ps)

    # ---- phase 1: ET_j = exp(s^T chunk) + colsum via accum, rhs tiles
    ets = []
    rhss = []

    def phase1(jt):
        et_t = et_pool.tile([P, L], bf16, tag="et")
        cs2 = stats.tile([P, 2], f32)
        for h in range(2):
            ps = psA.tile([P, 1024], f32, tag="p1")
            for q in range(2):
                isl = h * 2 + q
                nc.tensor.matmul(
                    ps[:, q * 512 : (q + 1) * 512],
                    lhsT=bT[:, jt * P : (jt + 1) * P],
                    rhs=aT[:, isl * 512 : (isl + 1) * 512],
                    start=True,
                    stop=True,
                )
            nc.scalar.activation(
                out=et_t[:, h * 1024 : (h + 1) * 1024],
                in_=ps,
                func=Exp,
                bias=neg_shift,
                scale=1.0,
                accum_out=cs2[:, h : h + 1],
            )
        csum = stats.tile([P, 1], f32)
        rcol = stats.tile([P, 1], f32)
        nc.vector.tensor_add(csum, cs2[:, 0:1], cs2[:, 1:2])
        nc.vector.reciprocal(rcol, csum)
        rhs_t = rhs_pool.tile([P, N2], bf16, tag="rhs")
        nc.vector.tensor_copy(out=rhs_t[:, 0:D], in_=b_nat[:, jt, :])
        nc.vector.memset(rhs_t[:, D : D + 1], 1.0)
        nc.vector.tensor_scalar_mul(
            out=rhs_t[:, D + 1 : N2], in0=a_nat[:, jt, :], scalar1=rcol
        )
        ets.append(et_t)
        rhss.append(rhs_t)

    for jt in range(NT):
        phase1(jt)

    # ---- phase 2a: first-half partials (j chunks 0..NH-1), spilled to SBUF.
    # Emitted after the full phase-1 loop so phase-1 matmuls keep PE priority;
    # these fill PE idle time while ScalarE works through the exps.
    parts = []
    for it in range(NT):
        po = psB.tile([P, N2], f32, tag="p2")
        for jt in range(NH):
            nc.tensor.matmul(
                po,
                lhsT=ets[jt][:, it * P : (it + 1) * P],
                rhs=rhss[jt],
                start=(jt == 0),
                stop=(jt == NH - 1),
            )
        pa = part_pool.tile([P, N2], f32, tag="pa")
        nc.vector.tensor_copy(out=pa, in_=po)
        parts.append(pa)

    # ---- phase 2b: second-half accumulation + epilogue per chunk
    for it in range(NT):
        po = psB.tile([P, N2], f32, tag="p2")
        for jt in range(NH, NT):
            nc.tensor.matmul(
                po,
                lhsT=ets[jt][:, it * P : (it + 1) * P],
                rhs=rhss[jt],
                start=(jt == NH),
                stop=(jt == NT - 1),
            )
        tot = sum_pool.tile([P, N2], f32, tag="tot")
        nc.vector.tensor_add(tot, po, parts[it])
        rrow = stats.tile([P, 1], f32)
        nc.vector.reciprocal(rrow, tot[:, D : D + 1])
        mab = out_pool.tile([P, 2, 3 * D], f32, tag="mab")
        ma_t = mab[:, 0, :]
        mb_t = mab[:, 1, :]
        # a_ = a_unnorm / rowsum ; b_ comes out normalized already
        nc.scalar.activation(out=ma_t[:, 0:D], in_=tot[:, 0:D], func=Copy, bias=0.0, scale=rrow)
        nc.scalar.activation(out=mb_t[:, 0:D], in_=tot[:, D + 1 : N2], func=Copy, bias=0.0, scale=1.0)
        # spread the derived columns: subs on DVE, muls on the idle GpSimd
        nc.vector.tensor_sub(ma_t[:, D : 2 * D], a_nat[:, it, :], ma_t[:, 0:D])
        nc.gpsimd.tensor_mul(ma_t[:, 2 * D : 3 * D], a_nat[:, it, :], ma_t[:, 0:D])
        nc.vector.tensor_sub(mb_t[:, D : 2 * D], b_nat[:, it, :], mb_t[:, 0:D])
        nc.gpsimd.tensor_mul(mb_t[:, 2 * D : 3 * D], b_nat[:, it, :], mb_t[:, 0:D])
        # one DMA for both planes: out[:, rows, D:4D] <- [ma | mb]
        nc.sync.dma_start(out=o_dram[:, it * P : (it + 1) * P, D : 4 * D], in_=mab)


def _build_nc():
    import concourse.bacc as bacc
    import concourse.mybir as mybir
    import concourse.tile as tile

    f32 = mybir.dt.float32
    nc = bacc.Bacc("TRN2", target_bir_lowering=False, debug=False, num_devices=B)
    a_dram = nc.dram_tensor("a", [L, D], f32, kind="ExternalInput").ap()
    b_dram = nc.dram_tensor("b", [L, D], f32, kind="ExternalInput").ap()
    o_dram = nc.dram_tensor("o", [2, L, 4 * D], f32, kind="ExternalOutput").ap()
    from contextlib import ExitStack

    with tile.TileContext(nc) as tc:
        with ExitStack() as ctx:
            _emit(ctx, tc, nc, a_dram, b_dram, o_dram)
    nc.finalize()
    return nc


def _get_nc():
    if "nc" not in _CACHE:
        _CACHE["nc"] = _build_nc()
    return _CACHE["nc"]


def kernel(a: np.ndarray, b: np.ndarray) -> np.ndarray:
    """Full inputs [8, 2048, 128] f32 -> full output [2, 8, 2048, 512] f32."""
    a = np.ascontiguousarray(a, dtype=np.float32)
    b = np.ascontiguousarray(b, dtype=np.float32)
    nc = _get_nc()
    from concourse import bass_utils

    in_maps = [{"a": a[c], "b": b[c]} for c in range(B)]
    res = bass_utils.run_bass_kernel_spmd(nc, in_maps, core_ids=list(range(B)))
    out = np.empty((2, B, L, 4 * D), dtype=np.float32)
    for c in range(B):
        out[:, c] = res.results[c]["o"]
    return out


# revision 10
# speedup vs baseline: 1.1136x; 1.0012x over previous
"""nn_LocalInference_58695023067411: batch-parallel Bass/Tile kernel, 8 NeuronCores.

Math per batch element (B=8, L=2048, D=128, one core per batch element):
  s  = a @ b.T                      # [L, L]
  a_ = softmax(s, axis=1) @ b       # row softmax
  b_ = softmax(s, axis=0) @ a       # col softmax
  out = [[a, a_, a-a_, a*a_], [b, b_, b-b_, b*b_]]   # [2, L, 4D]

Kernel strategy (per core):
  * Everything is computed from ET[j,i] = exp(s[i,j] - 50).  The constant
    shift is softmax-invariant on both axes and keeps exp() comfortably
    inside f32/bf16 range (max |s| ~ 84 on these inputs).
  * Phase 1: ET = exp(bT.T @ aT - 50) via PE matmuls (bf16 in, f32 PSUM),
    ScalarE exp PSUM->SBUF(bf16) with accum_out giving colsum_j for free.
    ScalarE is the pacing engine (~39us of exp); loads/transposes are
    ordered so the first exp can issue at ~4us.
  * Phase 2: one fused matmul per 128-row output chunk:
      psum[i, 0:2D+1] = sum_j ET[j,i] * [b | ones | a/colsum]_j
    giving unnormalized a_, rowsum_i, and b_ in one pass with ET stationary.
    The j-contraction is split in half: first-half partials run on the PE
    during the exp window (PSUM -> SBUF spill), second halves + epilogue
    run in the tail so the serial-after-last-exp work is halved.
  * Raw a/b columns of the output go out as dep-free HBM->HBM DMAs that fill
    the otherwise-idle DMA window during phase 1.

All matmul operands are bf16 (1 PE cycle/row vs 4 for f32); measured
end-to-end rel err vs the f32 reference is ~6e-3 (gate: 2e-2).
"""

import os
import sys

import numpy as np

sys.path.insert(0, "/opt/trn_rl_repo")

B, L, D = 8, 2048, 128
P = 128
NT = L // P          # 16 row/col chunks of 128
NH = NT // 2         # half-split of the phase-2 j contraction
C_SHIFT = 50.0       # subtracted inside exp; softmax-shift-invariant
N2 = 2 * D + 1       # [b | ones | a/colsum]

_CACHE = {}


def _emit(ctx, tc, nc, a_dram, b_dram, o_dram):
    import concourse.mybir as mybir
    from concourse.masks import make_identity

    f32 = mybir.dt.float32
    bf16 = mybir.dt.bfloat16
    Exp = mybir.ActivationFunctionType.Exp
    Copy = mybir.ActivationFunctionType.Copy

    persist = ctx.enter_context(tc.tile_pool(name="persist", bufs=1))
    et_pool = ctx.enter_context(tc.tile_pool(name="et", bufs=NT))
    rhs_pool = ctx.enter_context(tc.tile_pool(name="rhs", bufs=NT))
    part_pool = ctx.enter_context(tc.tile_pool(name="part", bufs=NT))
    stats = ctx.enter_context(tc.tile_pool(name="stats", bufs=4))
    sum_pool = ctx.enter_context(tc.tile_pool(name="sum", bufs=4))
    out_pool = ctx.enter_context(tc.tile_pool(name="outp", bufs=4))
    psA = ctx.enter_context(tc.tile_pool(name="psA", bufs=2, space="PSUM"))
    psB = ctx.enter_context(tc.tile_pool(name="psB", bufs=2, space="PSUM"))
    psT = ctx.enter_context(tc.tile_pool(name="psT", bufs=2, space="PSUM"))

    # ---- constants
    ident = persist.tile([P, P], f32)
    make_identity(nc, ident)
    neg_shift = persist.tile([P, 1], f32)
    nc.vector.memset(neg_shift, -C_SHIFT)

    # ---- load inputs: natural layout [p, t, d], split so the transposes
    # (and the first exps) start as early as possible.
    a_nat = persist.tile([P, NT, D], f32)
    b_nat = persist.tile([P, NT, D], f32)
    a_v = a_dram.rearrange("(t p) d -> p t d", p=P)
    b_v = b_dram.rearrange("(t p) d -> p t d", p=P)
    nc.sync.dma_start(out=b_nat[:, 0:4, :], in_=b_v[:, 0:4, :])
    nc.sync.dma_start(out=a_nat[:, 0:8, :], in_=a_v[:, 0:8, :])
    nc.sync.dma_start(out=b_nat[:, 4:8, :], in_=b_v[:, 4:8, :])
    nc.sync.dma_start(out=a_nat[:, 8:16, :], in_=a_v[:, 8:16, :])
    nc.sync.dma_start(out=b_nat[:, 8:16, :], in_=b_v[:, 8:16, :])

    # raw a/b occupy out[:, :, 0:D]; dep-free HBM->HBM copies that fill the
    # DMA-idle window during phase 1.
    nc.sync.dma_start(out=o_dram[0, :, 0:D], in_=a_dram)
    nc.sync.dma_start(out=o_dram[1, :, 0:D], in_=b_dram)

    # ---- transpose a, b to [d, i] layout (bf16) via PE transposes, in
    # groups of 4 through a dedicated 1-bank psum pool.  Order matches the
    # dependency chain of the first exps: phase1(jt<4) needs bT[0:512] and
    # all of aT used by its first psum half (aT[0:1024]).
    aT = persist.tile([P, L], bf16)
    bT = persist.tile([P, L], bf16)
    tr_order = [
        (b_nat, bT, 0), (a_nat, aT, 0), (a_nat, aT, 1), (b_nat, bT, 1),
        (a_nat, aT, 2), (a_nat, aT, 3), (b_nat, bT, 2), (b_nat, bT, 3),
    ]
    for src, dstT, g in tr_order:
        ps = psT.tile([P, 4 * P], f32, tag="tr")
        for k in range(4):
            t = g * 4 + k
            nc.tensor.matmul(
                ps[:, k * P : (k + 1) * P],
                lhsT=src[:, t, :],
                rhs=ident,
                is_transpose=True,
                start=True,
                stop=True,
            )
        nc.vector.tensor_copy(out=dstT[:, g * 512 : (g + 1) * 512], in_=ps)

    # ---- phase 1: ET_j = exp(s^T chunk) + colsum via accum, rhs tiles
    ets = []
    rhss = []

    def phase1(jt):
        et_t = et_pool.tile([P, L], bf16, tag="et")
        cs2 = stats.tile([P, 2], f32)
        for h in range(2):
            ps = psA.tile([P, 1024], f32, tag="p1")
            for q in range(2):
                isl = h * 2 + q
                nc.tensor.matmul(
                    ps[:, q * 512 : (q + 1) * 512],
                    lhsT=bT[:, jt * P : (jt + 1) * P],
                    rhs=aT[:, isl * 512 : (isl + 1) * 512],
                    start=True,
                    stop=True,
                )
            nc.scalar.activation(
                out=et_t[:, h * 1024 : (h + 1) * 1024],
                in_=ps,
                func=Exp,
                bias=neg_shift,
                scale=1.0,
                accum_out=cs2[:, h : h + 1],
            )
        csum = stats.tile([P, 1], f32)
        rcol = stats.tile([P, 1], f32)
        nc.vector.tensor_add(csum, cs2[:, 0:1], cs2[:, 1:2])
        nc.vector.reciprocal(rcol, csum)
        rhs_t = rhs_pool.tile([P, N2], bf16, tag="rhs")
        nc.vector.tensor_copy(out=rhs_t[:, 0:D], in_=b_nat[:, jt, :])
        nc.vector.memset(rhs_t[:, D : D + 1], 1.0)
        nc.vector.tensor_scalar_mul(
            out=rhs_t[:, D + 1 : N2], in0=a_nat[:, jt, :], scalar1=rcol
        )
        ets.append(et_t)
        rhss.append(rhs_t)

    for jt in range(NT):
        phase1(jt)

    # ---- phase 2a: first-half partials (j chunks 0..NH-1), spilled to SBUF.
    # Emitted after the full phase-1 loop so phase-1 matmuls keep PE priority;
    # these fill PE idle time while ScalarE works through the exps.
    parts = []
    for it in range(NT):
        po = psB.tile([P, N2], f32, tag="p2")
        for jt in range(NH):
            nc.tensor.matmul(
                po,
                lhsT=ets[jt][:, it * P : (it + 1) * P],
                rhs=rhss[jt],
                start=(jt == 0),
                stop=(jt == NH - 1),
            )
        pa = part_pool.tile([P, N2], f32, tag="pa")
        nc.vector.tensor_copy(out=pa, in_=po)
        parts.append(pa)

    # ---- phase 2b: second-half accumulation + epilogue per chunk
    for it in range(NT):
        po = psB.tile([P, N2], f32, tag="p2")
        for jt in range(NH, NT):
            nc.tensor.matmul(
                po,
                lhsT=ets[jt][:, it * P : (it + 1) * P],
                rhs=rhss[jt],
                start=(jt == NH),
                stop=(jt == NT - 1),
            )
        tot = sum_pool.tile([P, N2], f32, tag="tot")
        nc.vector.tensor_add(tot, po, parts[it])
        rrow = stats.tile([P, 1], f32)
        nc.vector.reciprocal(rrow, tot[:, D : D + 1])
        mab = out_pool.tile([P, 2, 3 * D], f32, tag="mab")
        ma_t = mab[:, 0, :]
        mb_t = mab[:, 1, :]
        # a_ = a_unnorm / rowsum ; b_ comes out normalized already
        nc.scalar.activation(out=ma_t[:, 0:D], in_=tot[:, 0:D], func=Copy, bias=0.0, scale=rrow)
        nc.scalar.activation(out=mb_t[:, 0:D], in_=tot[:, D + 1 : N2], func=Copy, bias=0.0, scale=1.0)
        # spread the derived columns: subs on DVE, muls on the idle GpSimd
        nc.vector.tensor_sub(ma_t[:, D : 2 * D], a_nat[:, it, :], ma_t[:, 0:D])
        nc.gpsimd.tensor_mul(ma_t[:, 2 * D : 3 * D], a_nat[:, it, :], ma_t[:, 0:D])
        nc.vector.tensor_sub(mb_t[:, D : 2 * D], b_nat[:, it, :], mb_t[:, 0:D])
        nc.gpsimd.tensor_mul(mb_t[:, 2 * D : 3 * D], b_nat[:, it, :], mb_t[:, 0:D])
        # one DMA for both planes: out[:, rows, D:4D] <- [ma | mb]
        nc.sync.dma_start(
            out=o_dram[:, it * P : (it + 1) * P, D : 4 * D].rearrange("c p d -> p c d"),
            in_=mab,
        )


def _build_nc():
    import concourse.bacc as bacc
    import concourse.mybir as mybir
    import concourse.tile as tile

    f32 = mybir.dt.float32
    nc = bacc.Bacc("TRN2", target_bir_lowering=False, debug=False, num_devices=B)
    a_dram = nc.dram_tensor("a", [L, D], f32, kind="ExternalInput").ap()
    b_dram = nc.dram_tensor("b", [L, D], f32, kind="ExternalInput").ap()
    o_dram = nc.dram_tensor("o", [2, L, 4 * D], f32, kind="ExternalOutput").ap()
    from contextlib import ExitStack

    with tile.TileContext(nc) as tc:
        with ExitStack() as ctx:
            _emit(ctx, tc, nc, a_dram, b_dram, o_dram)
    nc.finalize()
    return nc


def _get_nc():
    if "nc" not in _CACHE:
        _CACHE["nc"] = _build_nc()
    return _CACHE["nc"]


def kernel(a: np.ndarray, b: np.ndarray) -> np.ndarray:
    """Full inputs [8, 2048, 128] f32 -> full output [2, 8, 2048, 512] f32."""
    a = np.ascontiguousarray(a, dtype=np.float32)
    b = np.ascontiguousarray(b, dtype=np.float32)
    nc = _get_nc()
    from concourse import bass_utils

    in_maps = [{"a": a[c], "b": b[c]} for c in range(B)]
    res = bass_utils.run_bass_kernel_spmd(nc, in_maps, core_ids=list(range(B)))
    out = np.empty((2, B, L, 4 * D), dtype=np.float32)
    for c in range(B):
        out[:, c] = res.results[c]["o"]
    return out


# revision 15
# speedup vs baseline: 1.1238x; 1.0092x over previous
"""nn_LocalInference_58695023067411: batch-parallel Bass/Tile kernel, 8 NeuronCores.

Math per batch element (B=8, L=2048, D=128, one core per batch element):
  s  = a @ b.T                      # [L, L]
  a_ = softmax(s, axis=1) @ b       # row softmax
  b_ = softmax(s, axis=0) @ a       # col softmax
  out = [[a, a_, a-a_, a*a_], [b, b_, b-b_, b*b_]]   # [2, L, 4D]

Kernel strategy (per core):
  * Everything is computed from ET[j,i] = exp(s[i,j] - 50).  The constant
    shift is softmax-invariant on both axes and keeps exp() comfortably
    inside f32/bf16 range (max |s| ~ 84 on these inputs).
  * Phase 1: ET = exp(bT.T @ aT - 50) via PE matmuls (bf16 in, f32 PSUM),
    ScalarE exp PSUM->SBUF(bf16) with accum_out giving colsum_j for free.
    ScalarE is the pacing engine (~39us of exp); loads/transposes are
    ordered so the first exp can issue at ~4us.
  * Phase 2: one fused matmul per 128-row output chunk:
      psum[i, 0:2D+1] = sum_j ET[j,i] * [b | ones | a/colsum]_j
    giving unnormalized a_, rowsum_i, and b_ in one pass with ET stationary.
    The j-contraction is split in half: first-half partials run on the PE
    during the exp window (PSUM -> SBUF spill), second halves + epilogue
    run in the tail so the serial-after-last-exp work is halved.
  * Raw a/b columns of the output go out as dep-free HBM->HBM DMAs that fill
    the otherwise-idle DMA window during phase 1.

All matmul operands are bf16 (1 PE cycle/row vs 4 for f32); measured
end-to-end rel err vs the f32 reference is ~6e-3 (gate: 2e-2).
"""

import os
import sys

import numpy as np

sys.path.insert(0, "/opt/trn_rl_repo")

B, L, D = 8, 2048, 128
P = 128
NT = L // P          # 16 row/col chunks of 128
NH = NT // 2         # half-split of the phase-2 j contraction
C_SHIFT = 50.0       # subtracted inside exp; softmax-shift-invariant
N2 = 2 * D + 1       # [b | ones | a/colsum]

_CACHE = {}


def _emit(ctx, tc, nc, a_dram, b_dram, o_dram):
    import concourse.mybir as mybir
    from concourse.masks import make_identity

    f32 = mybir.dt.float32
    bf16 = mybir.dt.bfloat16
    Exp = mybir.ActivationFunctionType.Exp
    Copy = mybir.ActivationFunctionType.Copy

    persist = ctx.enter_context(tc.tile_pool(name="persist", bufs=1))
    et_pool = ctx.enter_context(tc.tile_pool(name="et", bufs=NT))
    rhs_pool = ctx.enter_context(tc.tile_pool(name="rhs", bufs=NT))
    part_pool = ctx.enter_context(tc.tile_pool(name="part", bufs=NT))
    stats = ctx.enter_context(tc.tile_pool(name="stats", bufs=4))
    sum_pool = ctx.enter_context(tc.tile_pool(name="sum", bufs=4))
    out_pool = ctx.enter_context(tc.tile_pool(name="outp", bufs=4))
    psA = ctx.enter_context(tc.tile_pool(name="psA", bufs=2, space="PSUM"))
    psB = ctx.enter_context(tc.tile_pool(name="psB", bufs=2, space="PSUM"))
    psT = ctx.enter_context(tc.tile_pool(name="psT", bufs=2, space="PSUM"))

    # ---- constants
    ident = persist.tile([P, P], f32)
    make_identity(nc, ident)
    neg_shift = persist.tile([P, 1], f32)
    nc.vector.memset(neg_shift, -C_SHIFT)

    # ---- load inputs: natural layout [p, t, d], split so the transposes
    # (and the first exps) start as early as possible.
    a_nat = persist.tile([P, NT, D], f32)
    b_nat = persist.tile([P, NT, D], f32)
    a_v = a_dram.rearrange("(t p) d -> p t d", p=P)
    b_v = b_dram.rearrange("(t p) d -> p t d", p=P)
    nc.sync.dma_start(out=b_nat[:, 0:4, :], in_=b_v[:, 0:4, :])
    nc.sync.dma_start(out=a_nat[:, 0:8, :], in_=a_v[:, 0:8, :])
    nc.sync.dma_start(out=a_nat[:, 8:16, :], in_=a_v[:, 8:16, :])
    nc.sync.dma_start(out=b_nat[:, 4:8, :], in_=b_v[:, 4:8, :])
    nc.sync.dma_start(out=b_nat[:, 8:16, :], in_=b_v[:, 8:16, :])

    # raw a/b occupy out[:, :, 0:D]; dep-free HBM->HBM copies that fill the
    # DMA-idle window during phase 1.
    nc.sync.dma_start(out=o_dram[0, :, 0:D], in_=a_dram)
    nc.sync.dma_start(out=o_dram[1, :, 0:D], in_=b_dram)

    # ---- transpose a, b to [d, i] layout (bf16) via PE transposes, in
    # groups of 4 through a dedicated 1-bank psum pool.  Interleaved with
    # phase-1 emission below so the first exp issues as early as possible.
    aT = persist.tile([P, L], bf16)
    bT = persist.tile([P, L], bf16)

    def tr_group(src, dstT, g):
        ps = psT.tile([P, 4 * P], f32, tag="tr")
        for k in range(4):
            t = g * 4 + k
            nc.tensor.matmul(
                ps[:, k * P : (k + 1) * P],
                lhsT=src[:, t, :],
                rhs=ident,
                is_transpose=True,
                start=True,
                stop=True,
            )
        nc.vector.tensor_copy(out=dstT[:, g * 512 : (g + 1) * 512], in_=ps)

    # ---- phase 1: ET_j = exp(s^T chunk) + colsum via accum, rhs tiles
    ets = []
    rhss = []

    def phase1(jt):
        et_t = et_pool.tile([P, L], bf16, tag="et")
        cs2 = stats.tile([P, 2], f32)
        for h in range(2):
            ps = psA.tile([P, 1024], f32, tag="p1")
            for q in range(2):
                isl = h * 2 + q
                nc.tensor.matmul(
                    ps[:, q * 512 : (q + 1) * 512],
                    lhsT=bT[:, jt * P : (jt + 1) * P],
                    rhs=aT[:, isl * 512 : (isl + 1) * 512],
                    start=True,
                    stop=True,
                )
            nc.scalar.activation(
                out=et_t[:, h * 1024 : (h + 1) * 1024],
                in_=ps,
                func=Exp,
                bias=neg_shift,
                scale=1.0,
                accum_out=cs2[:, h : h + 1],
            )
        csum = stats.tile([P, 1], f32)
        rcol = stats.tile([P, 1], f32)
        nc.vector.tensor_add(csum, cs2[:, 0:1], cs2[:, 1:2])
        nc.vector.reciprocal(rcol, csum)
        rhs_t = rhs_pool.tile([P, N2], bf16, tag="rhs")
        nc.vector.tensor_copy(out=rhs_t[:, 0:D], in_=b_nat[:, jt, :])
        nc.vector.memset(rhs_t[:, D : D + 1], 1.0)
        nc.vector.tensor_scalar_mul(
            out=rhs_t[:, D + 1 : N2], in0=a_nat[:, jt, :], scalar1=rcol
        )
        ets.append(et_t)
        rhss.append(rhs_t)

    # Emission order = scheduler priority, and all writers of a tile region
    # must be emitted before its readers.  phase1(jt) reads all of aT but
    # only bT[jt*128:(jt+1)*128], so the later b transposes interleave.
    tr_group(b_nat, bT, 0)
    for g in range(4):
        tr_group(a_nat, aT, g)
    for jt in range(4):
        phase1(jt)
    tr_group(b_nat, bT, 1)
    for jt in range(4, 8):
        phase1(jt)
    tr_group(b_nat, bT, 2)
    tr_group(b_nat, bT, 3)
    for jt in range(8, NT):
        phase1(jt)

    # ---- phase 2a: first-half partials (j chunks 0..NH-1), spilled to SBUF.
    # Emitted after the full phase-1 loop so phase-1 matmuls keep PE priority;
    # these fill PE idle time while ScalarE works through the exps.  They
    # reuse the transpose psum slots (same tag), giving 2 extra banks once
    # the transposes have drained.
    parts = []
    for it in range(NT):
        pool, tag = (psT, "tr") if it % 2 == 0 else (psB, "p2")
        po = pool.tile([P, N2], f32, tag=tag)
        for jt in range(NH):
            nc.tensor.matmul(
                po,
                lhsT=ets[jt][:, it * P : (it + 1) * P],
                rhs=rhss[jt],
                start=(jt == 0),
                stop=(jt == NH - 1),
            )
        pa = part_pool.tile([P, N2], f32, tag="pa")
        nc.vector.tensor_copy(out=pa, in_=po)
        parts.append(pa)

    # ---- phase 2b: second-half accumulation + epilogue per chunk
    for it in range(NT):
        pool, tag = (psT, "tr") if it % 2 == 0 else (psB, "p2")
        po = pool.tile([P, N2], f32, tag=tag)
        for jt in range(NH, NT):
            nc.tensor.matmul(
                po,
                lhsT=ets[jt][:, it * P : (it + 1) * P],
                rhs=rhss[jt],
                start=(jt == NH),
                stop=(jt == NT - 1),
            )
        tot = sum_pool.tile([P, N2], f32, tag="tot")
        nc.vector.tensor_add(tot, po, parts[it])
        rrow = stats.tile([P, 1], f32)
        nc.vector.reciprocal(rrow, tot[:, D : D + 1])
        mab = out_pool.tile([P, 2, 3 * D], f32, tag="mab")
        ma_t = mab[:, 0, :]
        mb_t = mab[:, 1, :]
        # a_ = a_unnorm / rowsum ; b_ comes out normalized already
        nc.scalar.activation(out=ma_t[:, 0:D], in_=tot[:, 0:D], func=Copy, bias=0.0, scale=rrow)
        nc.scalar.activation(out=mb_t[:, 0:D], in_=tot[:, D + 1 : N2], func=Copy, bias=0.0, scale=1.0)
        # spread the derived columns: subs on DVE, muls on the idle GpSimd
        nc.vector.tensor_sub(ma_t[:, D : 2 * D], a_nat[:, it, :], ma_t[:, 0:D])
        nc.gpsimd.tensor_mul(ma_t[:, 2 * D : 3 * D], a_nat[:, it, :], ma_t[:, 0:D])
        nc.vector.tensor_sub(mb_t[:, D : 2 * D], b_nat[:, it, :], mb_t[:, 0:D])
        nc.gpsimd.tensor_mul(mb_t[:, 2 * D : 3 * D], b_nat[:, it, :], mb_t[:, 0:D])
        # one DMA for both planes: out[:, rows, D:4D] <- [ma | mb]
        nc.sync.dma_start(
            out=o_dram[:, it * P : (it + 1) * P, D : 4 * D].rearrange("c p d -> p c d"),
            in_=mab,
        )


def _build_nc():
    import concourse.bacc as bacc
    import concourse.mybir as mybir
    import concourse.tile as tile

    f32 = mybir.dt.float32
    nc = bacc.Bacc("TRN2", target_bir_lowering=False, debug=False, num_devices=B)
    a_dram = nc.dram_tensor("a", [L, D], f32, kind="ExternalInput").ap()
    b_dram = nc.dram_tensor("b", [L, D], f32, kind="ExternalInput").ap()
    o_dram = nc.dram_tensor("o", [2, L, 4 * D], f32, kind="ExternalOutput").ap()
    from contextlib import ExitStack

    with tile.TileContext(nc) as tc:
        with ExitStack() as ctx:
            _emit(ctx, tc, nc, a_dram, b_dram, o_dram)
    nc.finalize()
    return nc


def _get_nc():
    if "nc" not in _CACHE:
        _CACHE["nc"] = _build_nc()
    return _CACHE["nc"]


def kernel(a: np.ndarray, b: np.ndarray) -> np.ndarray:
    """Full inputs [8, 2048, 128] f32 -> full output [2, 8, 2048, 512] f32."""
    a = np.ascontiguousarray(a, dtype=np.float32)
    b = np.ascontiguousarray(b, dtype=np.float32)
    nc = _get_nc()
    from concourse import bass_utils

    in_maps = [{"a": a[c], "b": b[c]} for c in range(B)]
    res = bass_utils.run_bass_kernel_spmd(nc, in_maps, core_ids=list(range(B)))
    out = np.empty((2, B, L, 4 * D), dtype=np.float32)
    for c in range(B):
        out[:, c] = res.results[c]["o"]
    return out
